# revision 70
# baseline (speedup 1.0000x reference)
"""GQA attention kernel for Trainium2, 8-core tensor-parallel.

Sharding: core c handles batch b=c//4 and kv-head pair {2*(c%4), 2*(c%4)+1}
(8 q heads). q/k/v projections column-sharded, out_proj row-sharded; the
4 partial out_proj products per batch are summed on host (the gather).

Single software-pipelined instruction stream per core:
  - projections in 256-token chunks, rotating 2-bank PSUM (accum + V
    transpose share one pool tag), biases on the ACT engine;
  - attention chunk c0 (lq 0:1024) runs with projection chunks 4..7
    interleaved as PE filler; chunk c1 runs with out_proj token-chunk-0
    groups as filler; out_proj token-chunk-1 forms the tail;
  - scores [lk,lq] double-buffered in PSUM so exp (ACT) overlaps the
    score/PV matmuls (PE); causal trim at 256 granularity keeps every
    fp32r matmul free-dim >= 256 (full PE rate);
  - softmax denominator rides as a ones-column appended to V; no max
    subtraction (scores are O(1)); normalize = DVE copy/recip + Pool
    broadcast + DVE mul, deferred off the PV critical path.
"""
import sys
if "/opt/trn_rl_repo" not in sys.path:
    sys.path.insert(0, "/opt/trn_rl_repo")
import numpy as np

HID = 2048
L = 2048
D = 64
NCORE = 8
NKT = HID // 128        # 16 k-tiles over hidden
CHA = 256               # token chunk for projections
NCHA = L // CHA         # 8
LQC = 1024              # lq chunk for attention
BIG = -1e32

_cached = {}


def _build():
    import concourse.bass as bass
    from concourse import bacc
    import concourse.mybir as mybir
    import concourse.tile as tile

    F32R = mybir.dt.float32r
    F32 = mybir.dt.float32
    EXP = mybir.ActivationFunctionType.Exp
    IDENT = mybir.ActivationFunctionType.Identity

    nc = bacc.Bacc(None, target_bir_lowering=False)
    xT = nc.dram_tensor("xT", [128, NKT, L], F32R, kind="ExternalInput")
    qw = nc.dram_tensor("qw", [128, NKT, 512], F32R, kind="ExternalInput")
    kw = nc.dram_tensor("kw", [128, NKT, 128], F32R, kind="ExternalInput")
    vw = nc.dram_tensor("vw", [128, NKT, 128], F32R, kind="ExternalInput")
    ow = nc.dram_tensor("ow", [128, 4, HID], F32R, kind="ExternalInput")
    qb = nc.dram_tensor("qb", [128, 4], F32, kind="ExternalInput")
    kb = nc.dram_tensor("kb", [128, 1], F32, kind="ExternalInput")
    vb = nc.dram_tensor("vb", [128, 1], F32, kind="ExternalInput")
    tri1 = nc.dram_tensor("tri1", [128, 128], F32, kind="ExternalInput")
    tri2 = nc.dram_tensor("tri2", [128, 256], F32, kind="ExternalInput")
    ident = nc.dram_tensor("ident", [128, 128], F32R, kind="ExternalInput")
    outp = nc.dram_tensor("outp", [NKT, 128, L], F32, kind="ExternalOutput")

    with tile.TileContext(nc) as tc:
        with tc.tile_pool(name="cst", bufs=1) as cst, \
             tc.tile_pool(name="res", bufs=1) as res, \
             tc.tile_pool(name="wk", bufs=1) as wk:
            # x-chunk staging + vtmp are released manually after c0 so
            # their 33KB of SBUF can serve the out_proj staging buffers.
            xp = tc.alloc_tile_pool(name="xp", bufs=1, space="SBUF")
            vt = tc.alloc_tile_pool(name="vt", bufs=1, space="SBUF")
            tri1_sb = cst.tile([128, 128], F32)
            tri2_sb = cst.tile([128, 256], F32)
            idn = cst.tile([128, 128], F32R)
            qb_sb = cst.tile([128, 4], F32)
            kb_sb = cst.tile([128, 1], F32)
            vb_sb = cst.tile([128, 1], F32)
            qw_sb = res.tile([128, NKT, 512], F32R)
            kw_sb = res.tile([128, NKT, 128], F32R)
            vw_sb = res.tile([128, NKT, 128], F32R)

            x_tiles = {}

            def x_dma(n, split=1):
                # Only call once all readers of the reused x buffer
                # (chunk n-2's matmuls) have been emitted.
                x_sb = xp.tile([128, NKT, CHA], F32R, tag="x", bufs=2,
                               name="x_sb")
                step = NKT // split
                for s in range(split):
                    nc.sync.dma_start(
                        out=x_sb[:, s * step:(s + 1) * step, :],
                        in_=xT.ap()[:, s * step:(s + 1) * step,
                                    CHA * n:CHA * n + CHA])
                x_tiles[n] = x_sb

            # startup order tuned for the serial DMA device: first v-group
            # inputs (x0 first half + vw), then the rest, big qw last.
            x_sb0 = xp.tile([128, NKT, CHA], F32R, tag="x", bufs=2, name="x_sb0")
            nc.sync.dma_start(out=vw_sb[:, 0:4, :], in_=vw.ap()[:, 0:4, :])
            nc.sync.dma_start(out=x_sb0[:, 0:4, :], in_=xT.ap()[:, 0:4, 0:CHA])
            nc.sync.dma_start(out=vw_sb[:, 4:16, :], in_=vw.ap()[:, 4:16, :])
            nc.sync.dma_start(out=x_sb0[:, 4:8, :], in_=xT.ap()[:, 4:8, 0:CHA])
            nc.sync.dma_start(out=x_sb0[:, 8:16, :], in_=xT.ap()[:, 8:16, 0:CHA])
            nc.sync.dma_start(out=kw_sb, in_=kw.ap())
            x_tiles[0] = x_sb0
            # tiny consts (biases + idn) first; the tri masks are not
            # needed until attention starts, so they follow qw
            for dst, src in [(qb_sb, qb), (kb_sb, kb), (vb_sb, vb),
                             (idn, ident)]:
                nc.sync.dma_start(out=dst, in_=src.ap())
            x_dma(1, split=2)
            for mt in range(4):
                for kh in range(2):   # kt-halves so q-groups start earlier
                    nc.sync.dma_start(
                        out=qw_sb[:, 8 * kh:8 * kh + 8, 128 * mt:128 * mt + 128],
                        in_=qw.ap()[:, 8 * kh:8 * kh + 8, 128 * mt:128 * mt + 128])
            nc.sync.dma_start(out=tri1_sb, in_=tri1.ap())
            nc.sync.dma_start(out=tri2_sb, in_=tri2.ap())

            qT_sb = res.tile([128, 4, L], F32R)   # head h: parts 64*(h//4), tile h%4
            kT_sb = res.tile([128, L], F32R)      # kv j at parts 64j
            v_aug = res.tile([128, NKT, 130], F32R)
            yT_c0 = res.tile([128, 4, LQC], F32R)
            yT_c1 = res.tile([128, 4, LQC], F32R)
            yT_cs = [yT_c0, yT_c1]
            nc.vector.memset(v_aug[:, :, 64:65].bitcast(F32), 1.0)
            nc.vector.memset(v_aug[:, :, 129:130].bitcast(F32), 1.0)

            # ---------- helpers ----------
            def seg_info(c, t):
                """(o_seg, o_exact, masked) for lk-tile t of lq chunk c."""
                masked = 128 * t >= LQC * c
                if not masked:
                    return 0, 0, False
                oe = 128 * t - LQC * c
                return (oe // 256) * 256, oe, True

            def segs_of(o_seg):
                if o_seg < 512:
                    return [(o_seg, 512), (512, LQC)]
                return [(o_seg, LQC)]

            pend_norm = []  # deferred normalize tails

            def norm_part1(c, h, pv):
                y_sb = wk.tile([65, LQC], F32, tag="ysb", bufs=1, name="y_sb")
                nc.vector.tensor_copy(out=y_sb, in_=pv)
                pend_norm.append((c, h, y_sb))

            def norm_part2():
                if not pend_norm:
                    return
                c, h, y_sb = pend_norm.pop(0)
                mt = h % 4
                r = wk.tile([1, LQC], F32, tag="r", bufs=1, name="r")
                nc.vector.reciprocal(r, y_sb[64:65, :])
                bc = wk.tile([64, LQC], F32, tag="bc", bufs=1, name="bc")
                nc.gpsimd.partition_broadcast(bc, r)
                if h < 4:
                    nc.vector.tensor_mul(out=yT_cs[c][0:64, mt, :],
                                         in0=y_sb[0:64, :], in1=bc)
                else:
                    yt = wk.tile([64, LQC], F32R, tag="yt", bufs=1, name="yt")
                    nc.vector.tensor_mul(out=yt, in0=y_sb[0:64, :], in1=bc)
                    nc.sync.dma_start(out=yT_cs[c][64:128, mt, :], in_=yt)

            P = {}   # late-bound tile pools: 'sc', 'pv', 'pp'

            def att_head(c, h, fillers, fill_points):
                scp, pvp = P['sc'], P['pv']
                base, mt, j = 64 * (h // 4), h % 4, h // 4
                ntile = 8 * (c + 1)
                pv = pvp.tile([65, LQC], F32, tag="pv", name="pv")
                pend_pv = []   # lag-2 software pipeline: pv trails scores
                for t in range(ntile):
                    os_, oe, masked = seg_info(c, t)
                    segs = segs_of(os_)
                    sc_t = scp.tile([128, LQC], F32, tag="sc", name="sc_t")
                    for (a, b) in segs:
                        nc.tensor.matmul(
                            sc_t[:, a:b],
                            kT_sb[base:base + 64, 128 * t:128 * t + 128],
                            qT_sb[base:base + 64, mt, LQC * c + a:LQC * c + b],
                            start=True, stop=True)
                    # causal mask: multiplicative 0/1 keep-mask on Pool,
                    # applied POST-exp on expS (SBUF — gpsimd cannot touch
                    # PSUM). Keeps the score->exp chain two-engine so ACT
                    # never waits on a mask op; the PV matmul eats the mask
                    # latency where the PE has filler slack.
                    ex = wk.tile([128, LQC], F32R, tag="expS", bufs=3, name="ex")
                    nc.scalar.activation(out=ex[:, os_:LQC], in_=sc_t[:, os_:LQC],
                                         func=EXP, scale=0.125)
                    if masked:
                        if oe == os_:
                            nc.gpsimd.tensor_mul(out=ex[:, oe:oe + 128],
                                                 in0=ex[:, oe:oe + 128],
                                                 in1=tri1_sb)
                        else:
                            nc.gpsimd.tensor_mul(out=ex[:, os_:os_ + 256],
                                                 in0=ex[:, os_:os_ + 256],
                                                 in1=tri2_sb)
                    pend_pv.append((ex, segs, t))
                    if len(pend_pv) > 2:
                        pex, psegs, pt = pend_pv.pop(0)
                        for (a, b) in psegs:
                            nc.tensor.matmul(pv[:, a:b],
                                             v_aug[:, pt, 65 * j:65 * j + 65],
                                             pex[:, a:b],
                                             start=(pt == 0), stop=False)
                    if t == 0:
                        norm_part2()
                    if t in fill_points and fillers:
                        fillers.pop(0)()
                while pend_pv:
                    pex, psegs, pt = pend_pv.pop(0)
                    for (a, b) in psegs:
                        nc.tensor.matmul(pv[:, a:b],
                                         v_aug[:, pt, 65 * j:65 * j + 65],
                                         pex[:, a:b],
                                         start=(pt == 0),
                                         stop=(pt == ntile - 1))
                norm_part1(c, h, pv)

            # ---- Phase A projections + c0/c1 attention + out_proj ----
            pend_tr = []

            def proj_group(n, f, bias_dve=False):
                """f: 0=k, 1=v, 2..5 = q tile f-2. 16 matmuls + bias.
                V transposes are deferred to the next group so the PE
                never waits on the bias-add. bias_dve routes the bias to
                DVE (used for c0-filler chunks where ACT is exp-bound)."""
                pp = P['pp']
                x_sb = x_tiles[n]
                sl = slice(CHA * n, CHA * n + CHA)
                ps = pp.tile([128, 512], F32, tag="pp", name="ps")
                for kt in range(NKT):
                    if f == 0:
                        stat = kw_sb[:, kt, :]
                    elif f == 1:
                        stat = vw_sb[:, kt, :]
                    else:
                        mt = f - 2
                        stat = qw_sb[:, kt, 128 * mt:128 * mt + 128]
                    nc.tensor.matmul(ps[:, 0:CHA], stat, x_sb[:, kt, :],
                                     start=(kt == 0), stop=(kt == NKT - 1))

                def bias_add(out_ap, bias_ap):
                    if bias_dve:
                        nc.vector.tensor_scalar_add(out=out_ap, in0=ps[:, 0:CHA],
                                                    scalar1=bias_ap)
                    else:
                        nc.scalar.activation(out=out_ap, in_=ps[:, 0:CHA],
                                             func=IDENT, bias=bias_ap, scale=1.0)
                if f == 0:
                    bias_add(kT_sb[:, sl], kb_sb)
                elif f >= 2:
                    mt = f - 2
                    bias_add(qT_sb[:, mt, sl], qb_sb[:, mt:mt + 1])
                else:
                    vtmp = vt.tile([128, CHA], F32R, tag="vt", name="vtmp")
                    bias_add(vtmp, vb_sb)
                while pend_tr:
                    pend_tr.pop(0)()
                if f == 1:
                    vtmp_ = vtmp
                    for tt in range(CHA // 128):
                        def tr(n=n, tt=tt, vtmp=vtmp_):
                            trpool = P['pp']
                            ti = (CHA * n) // 128 + tt
                            trp = trpool.tile([128, 512], F32R, tag="pp",
                                              name="trp")
                            nc.tensor.transpose(
                                trp[:, 0:128],
                                vtmp[:, 128 * tt:128 * tt + 128], idn)
                            nc.vector.tensor_copy(out=v_aug[:, ti, 0:64],
                                                  in_=trp[:, 0:64])
                            nc.vector.tensor_copy(out=v_aug[:, ti, 65:129],
                                                  in_=trp[:, 64:128])
                        pend_tr.append(tr)

            order = [1, 0, 2, 3, 4, 5]   # v first, then k, then q tiles

            # projections for tokens 0:1024 (chunks 0..3) in a dedicated
            # 4-bank PSUM pool: deep rotation keeps the PE stream gapless
            # so the p-state ramp reaches full clock. v/k groups of chunks
            # 0-1 run first so the PE is never waiting on the large qw DMA.
            with tc.tile_pool(name="ppA", bufs=4, space="PSUM") as ppA:
                P['pp'] = ppA
                for n in (0, 1):
                    proj_group(n, 1)
                    proj_group(n, 0)
                for f in (2, 3, 4, 5):
                    proj_group(0, f)
                x_dma(2)
                for f in (2, 3, 4, 5):
                    proj_group(1, f)
                proj_group(2, 1)
                proj_group(2, 0)
                x_dma(3)
                for f in (2, 3, 4, 5):
                    proj_group(2, f)
                proj_group(3, 1)
                proj_group(3, 0)
                x_dma(4)
                for f in (2, 3, 4, 5):
                    proj_group(3, f)
                x_dma(5)
                while pend_tr:
                    pend_tr.pop(0)()

            scp = tc.alloc_tile_pool(name="scp", bufs=2, space="PSUM")
            pvp = tc.alloc_tile_pool(name="pvp", bufs=1, space="PSUM")
            P['sc'], P['pv'] = scp, pvp

            # c0 attention with projection chunks 4..7 as PE filler
            # (filler biases on DVE: ACT is exp-bound during attention)
            with tc.tile_pool(name="ppB", bufs=2, space="PSUM") as ppB:
                P['pp'] = ppB
                fillers = []
                for n in range(4, NCHA):
                    for i, f in enumerate(order):
                        def g(n=n, f=f, last=(i == len(order) - 1)):
                            proj_group(n, f, bias_dve=True)
                            if last and n + 2 < NCHA:
                                x_dma(n + 2)
                        fillers.append(g)
                # heads 4-7 first: their normalize needs an extra shift
                # DMA, so the LAST heads' (0-3) normalize chains are short
                # and the downstream out_proj starts sooner
                for h in (4, 5, 6, 7, 0, 1, 2, 3):
                    att_head(0, h, fillers, fill_points=(1, 3, 5))
                while fillers:
                    fillers.pop(0)()
                while pend_tr:
                    pend_tr.pop(0)()
            vt.release()
            xp.release()

            # ---- c1 attention with out_proj chunk-0 filler; tail ----
            owp = tc.alloc_tile_pool(name="owp", bufs=1, space="SBUF")
            od = tc.alloc_tile_pool(name="od", bufs=1, space="SBUF")
            opsC1 = tc.alloc_tile_pool(name="opsC1", bufs=2, space="PSUM")
            P['ops'] = opsC1

            ow_tiles = {}

            def ow_dma(ot):
                ow_t = owp.tile([128, 4, 128], F32R, tag="ow", bufs=16,
                                name="ow_t")
                nc.sync.dma_start(out=ow_t,
                                  in_=ow.ap()[:, :, 128 * ot:128 * ot + 128])
                ow_tiles[ot] = ow_t

            def d_group(ot, half, pool_copy=False, split_store=False):
                """8 matmuls + 2 psum copies + ONE [128,1024] store
                for output token range half*1024 : half*1024+1024.
                split_store streams each 512-half as soon as it is copied
                (used for the final groups to shorten the drain tail)."""
                ow_t = ow_tiles[ot]
                o_sb = od.tile([128, 1024], F32, tag="osb", bufs=4,
                               name="o_sb")
                for sub in range(2):
                    o_ps = P['ops'].tile([128, 512], F32, tag="ops",
                                         name="o_ps")
                    for it in range(4):
                        nc.tensor.matmul(
                            o_ps, ow_t[:, it, :],
                            yT_cs[half][:, it, 512 * sub:512 * sub + 512],
                            start=(it == 0), stop=(it == 3))
                    # gpsimd can't read PSUM; ACT is idle in the tail, so
                    # alternate the psum drains between DVE and ACT there
                    if pool_copy and sub == 1:
                        nc.scalar.copy(out=o_sb[:, 512 * sub:512 * sub + 512],
                                       in_=o_ps)
                    else:
                        nc.vector.tensor_copy(
                            out=o_sb[:, 512 * sub:512 * sub + 512], in_=o_ps)
                    if split_store:
                        nc.sync.dma_start(
                            out=outp.ap()[ot, :,
                                          1024 * half + 512 * sub:
                                          1024 * half + 512 * sub + 512],
                            in_=o_sb[:, 512 * sub:512 * sub + 512])
                if not split_store:
                    nc.sync.dma_start(
                        out=outp.ap()[ot, :, 1024 * half:1024 * half + 1024],
                        in_=o_sb)

            fillers = []
            for ot in range(NKT):
                def g0(ot=ot):
                    if ot == 0:
                        ow_dma(0)
                        ow_dma(1)
                    if ot + 2 < NKT:
                        ow_dma(ot + 2)
                    d_group(ot, 0)
                fillers.append(g0)
            for h in (4, 5, 6, 7, 0, 1, 2, 3):
                att_head(1, h, fillers, fill_points=(1, 9))
            while fillers:
                fillers.pop(0)()
            norm_part2()
            norm_part2()

            # tail: attention PSUM pools are done — hand their 6 banks to
            # a deep out_proj accumulation pool so nothing rotates-stalls
            opsC1.release()
            pvp.release()
            scp.release()
            opsD2 = tc.alloc_tile_pool(name="opsD2", bufs=6, space="PSUM")
            P['ops'] = opsD2
            for ot in range(NKT):
                d_group(ot, 1, pool_copy=True, split_store=(ot >= NKT - 2))
            opsD2.release()
            od.release()
            owp.release()
    nc.compile()
    return nc


def _perm512():
    p = np.empty(512, dtype=np.int64)
    for mt in range(4):
        for half in range(2):
            head = mt + 4 * half
            p[128 * mt + 64 * half:128 * mt + 64 * half + 64] = \
                np.arange(64 * head, 64 * head + 64)
    return p


def kernel(x, attention_mask, q_w, q_b, k_w, k_b, v_w, v_b, o_w, o_b):
    from concourse.bass_utils import run_bass_kernel_spmd

    x = np.asarray(x, dtype=np.float32)
    q_w = np.asarray(q_w, dtype=np.float32); q_b = np.asarray(q_b, dtype=np.float32)
    k_w = np.asarray(k_w, dtype=np.float32); k_b = np.asarray(k_b, dtype=np.float32)
    v_w = np.asarray(v_w, dtype=np.float32); v_b = np.asarray(v_b, dtype=np.float32)
    o_w = np.asarray(o_w, dtype=np.float32); o_b = np.asarray(o_b, dtype=np.float32)
    am = np.asarray(attention_mask)
    assert am.all(), "kernel assumes attention_mask == all ones"

    if "nc" not in _cached:
        _cached["nc"] = _build()
    nc = _cached["nc"]

    perm = _perm512()
    # 0/1 keep-masks applied post-exp: 0 where lk > lq (causal), 1 elsewhere
    causal = np.arange(128)[:, None] > np.arange(128)[None, :]
    tri1_np = np.where(causal, np.float32(0), np.float32(1)).astype(np.float32)
    tri2_np = np.concatenate(
        [np.zeros((128, 128), dtype=np.float32), tri1_np], axis=1)
    id_np = np.eye(128, dtype=np.float32)

    in_maps = []
    for c in range(NCORE):
        b, g = c // 4, c % 4
        G0 = 512 * g
        xT_t = np.ascontiguousarray(
            x[b].T.reshape(NKT, 128, L).transpose(1, 0, 2))
        qws = q_w[G0:G0 + 512][perm]
        qw_t = np.ascontiguousarray(qws.T.reshape(NKT, 128, 512).transpose(1, 0, 2))
        kws = k_w[128 * g:128 * g + 128]
        kw_t = np.ascontiguousarray(kws.T.reshape(NKT, 128, 128).transpose(1, 0, 2))
        vws = v_w[128 * g:128 * g + 128]
        vw_t = np.ascontiguousarray(vws.T.reshape(NKT, 128, 128).transpose(1, 0, 2))
        owp = o_w[:, G0:G0 + 512][:, perm]
        ow_t = np.ascontiguousarray(owp.T.reshape(4, 128, HID).transpose(1, 0, 2))
        qb_t = np.ascontiguousarray(q_b[G0:G0 + 512][perm].reshape(4, 128).T)
        kb_t = k_b[128 * g:128 * g + 128].reshape(128, 1).copy()
        vb_t = v_b[128 * g:128 * g + 128].reshape(128, 1).copy()
        in_maps.append({"xT": xT_t, "qw": qw_t, "kw": kw_t, "vw": vw_t,
                        "ow": ow_t, "qb": qb_t, "kb": kb_t, "vb": vb_t,
                        "tri1": tri1_np, "tri2": tri2_np, "ident": id_np})

    res = run_bass_kernel_spmd(nc, in_maps, core_ids=list(range(NCORE)))
    out = np.empty((2, L, HID), dtype=np.float32)
    for b in range(2):
        acc = res.results[4 * b]["outp"].astype(np.float32).copy()
        for i in range(1, 4):
            acc += res.results[4 * b + i]["outp"]
        out[b] = acc.reshape(HID, L).T + o_b
    return out


# revision 71
# speedup vs baseline: 1.0064x; 1.0064x over previous
"""GQA attention kernel for Trainium2, 8-core tensor-parallel.

Sharding: core c handles batch b=c//4 and kv-head pair {2*(c%4), 2*(c%4)+1}
(8 q heads). q/k/v projections column-sharded, out_proj row-sharded; the
4 partial out_proj products per batch are summed on host (the gather).

Single software-pipelined instruction stream per core:
  - projections in 256-token chunks, rotating 2-bank PSUM (accum + V
    transpose share one pool tag), biases on the ACT engine;
  - attention chunk c0 (lq 0:1024) runs with projection chunks 4..7
    interleaved as PE filler; chunk c1 runs with out_proj token-chunk-0
    groups as filler; out_proj token-chunk-1 forms the tail;
  - scores [lk,lq] double-buffered in PSUM so exp (ACT) overlaps the
    score/PV matmuls (PE); causal trim at 256 granularity keeps every
    fp32r matmul free-dim >= 256 (full PE rate);
  - softmax denominator rides as a ones-column appended to V; no max
    subtraction (scores are O(1)); normalize = DVE copy/recip + Pool
    broadcast + DVE mul, deferred off the PV critical path.
"""
import sys
if "/opt/trn_rl_repo" not in sys.path:
    sys.path.insert(0, "/opt/trn_rl_repo")
import numpy as np

HID = 2048
L = 2048
D = 64
NCORE = 8
NKT = HID // 128        # 16 k-tiles over hidden
CHA = 256               # token chunk for projections
NCHA = L // CHA         # 8
LQC = 1024              # lq chunk for attention
BIG = -1e32

_cached = {}


def _build():
    import concourse.bass as bass
    from concourse import bacc
    import concourse.mybir as mybir
    import concourse.tile as tile

    F32R = mybir.dt.float32r
    F32 = mybir.dt.float32
    EXP = mybir.ActivationFunctionType.Exp
    IDENT = mybir.ActivationFunctionType.Identity

    nc = bacc.Bacc(None, target_bir_lowering=False)
    xT = nc.dram_tensor("xT", [128, NKT, L], F32R, kind="ExternalInput")
    qw = nc.dram_tensor("qw", [128, NKT, 512], F32R, kind="ExternalInput")
    kw = nc.dram_tensor("kw", [128, NKT, 128], F32R, kind="ExternalInput")
    vw = nc.dram_tensor("vw", [128, NKT, 128], F32R, kind="ExternalInput")
    ow = nc.dram_tensor("ow", [128, 4, HID], F32R, kind="ExternalInput")
    qb = nc.dram_tensor("qb", [128, 4], F32, kind="ExternalInput")
    kb = nc.dram_tensor("kb", [128, 1], F32, kind="ExternalInput")
    vb = nc.dram_tensor("vb", [128, 1], F32, kind="ExternalInput")
    tri1 = nc.dram_tensor("tri1", [128, 128], F32, kind="ExternalInput")
    tri2 = nc.dram_tensor("tri2", [128, 256], F32, kind="ExternalInput")
    ident = nc.dram_tensor("ident", [128, 128], F32R, kind="ExternalInput")
    outp = nc.dram_tensor("outp", [NKT, 128, L], F32, kind="ExternalOutput")

    with tile.TileContext(nc) as tc:
        with tc.tile_pool(name="cst", bufs=1) as cst, \
             tc.tile_pool(name="res", bufs=1) as res, \
             tc.tile_pool(name="wk", bufs=1) as wk:
            # x-chunk staging + vtmp are released manually after c0 so
            # their 33KB of SBUF can serve the out_proj staging buffers.
            xp = tc.alloc_tile_pool(name="xp", bufs=1, space="SBUF")
            vt = tc.alloc_tile_pool(name="vt", bufs=1, space="SBUF")
            tri1_sb = cst.tile([128, 128], F32)
            tri2_sb = cst.tile([128, 256], F32)
            idn = cst.tile([128, 128], F32R)
            qb_sb = cst.tile([128, 4], F32)
            kb_sb = cst.tile([128, 1], F32)
            vb_sb = cst.tile([128, 1], F32)
            qw_sb = res.tile([128, NKT, 512], F32R)
            kw_sb = res.tile([128, NKT, 128], F32R)
            vw_sb = res.tile([128, NKT, 128], F32R)

            x_tiles = {}

            def x_dma(n, split=4):
                # Only call once all readers of the reused x buffer
                # (chunk n-2's matmuls) have been emitted.
                x_sb = xp.tile([128, NKT, CHA], F32R, tag="x", bufs=2,
                               name="x_sb")
                step = NKT // split
                for s in range(split):
                    nc.sync.dma_start(
                        out=x_sb[:, s * step:(s + 1) * step, :],
                        in_=xT.ap()[:, s * step:(s + 1) * step,
                                    CHA * n:CHA * n + CHA])
                x_tiles[n] = x_sb

            # startup order tuned for the serial DMA device: first v-group
            # inputs (x0 first half + vw), then the rest, big qw last.
            x_sb0 = xp.tile([128, NKT, CHA], F32R, tag="x", bufs=2, name="x_sb0")
            nc.sync.dma_start(out=vw_sb[:, 0:4, :], in_=vw.ap()[:, 0:4, :])
            nc.sync.dma_start(out=x_sb0[:, 0:4, :], in_=xT.ap()[:, 0:4, 0:CHA])
            nc.sync.dma_start(out=vw_sb[:, 4:16, :], in_=vw.ap()[:, 4:16, :])
            nc.sync.dma_start(out=x_sb0[:, 4:8, :], in_=xT.ap()[:, 4:8, 0:CHA])
            nc.sync.dma_start(out=x_sb0[:, 8:16, :], in_=xT.ap()[:, 8:16, 0:CHA])
            nc.sync.dma_start(out=kw_sb, in_=kw.ap())
            x_tiles[0] = x_sb0
            # tiny consts (biases + idn) first; the tri masks are not
            # needed until attention starts, so they follow qw
            for dst, src in [(qb_sb, qb), (kb_sb, kb), (vb_sb, vb),
                             (idn, ident)]:
                nc.sync.dma_start(out=dst, in_=src.ap())
            x_dma(1, split=4)
            for mt in range(4):
                for kh in range(2):   # kt-halves so q-groups start earlier
                    nc.sync.dma_start(
                        out=qw_sb[:, 8 * kh:8 * kh + 8, 128 * mt:128 * mt + 128],
                        in_=qw.ap()[:, 8 * kh:8 * kh + 8, 128 * mt:128 * mt + 128])
            nc.sync.dma_start(out=tri1_sb, in_=tri1.ap())
            nc.sync.dma_start(out=tri2_sb, in_=tri2.ap())

            qT_sb = res.tile([128, 4, L], F32R)   # head h: parts 64*(h//4), tile h%4
            kT_sb = res.tile([128, L], F32R)      # kv j at parts 64j
            v_aug = res.tile([128, NKT, 130], F32R)
            yT_c0 = res.tile([128, 4, LQC], F32R)
            yT_c1 = res.tile([128, 4, LQC], F32R)
            yT_cs = [yT_c0, yT_c1]
            nc.vector.memset(v_aug[:, :, 64:65].bitcast(F32), 1.0)
            nc.vector.memset(v_aug[:, :, 129:130].bitcast(F32), 1.0)

            # ---------- helpers ----------
            def seg_info(c, t):
                """(o_seg, o_exact, masked) for lk-tile t of lq chunk c."""
                masked = 128 * t >= LQC * c
                if not masked:
                    return 0, 0, False
                oe = 128 * t - LQC * c
                return (oe // 256) * 256, oe, True

            def segs_of(o_seg):
                if o_seg < 512:
                    return [(o_seg, 512), (512, LQC)]
                return [(o_seg, LQC)]

            pend_norm = []  # deferred normalize tails

            def norm_part1(c, h, pv):
                y_sb = wk.tile([65, LQC], F32, tag="ysb", bufs=1, name="y_sb")
                nc.vector.tensor_copy(out=y_sb, in_=pv)
                pend_norm.append((c, h, y_sb))

            def norm_part2():
                if not pend_norm:
                    return
                c, h, y_sb = pend_norm.pop(0)
                mt = h % 4
                r = wk.tile([1, LQC], F32, tag="r", bufs=1, name="r")
                nc.vector.reciprocal(r, y_sb[64:65, :])
                bc = wk.tile([64, LQC], F32, tag="bc", bufs=1, name="bc")
                nc.gpsimd.partition_broadcast(bc, r)
                if h < 4:
                    nc.vector.tensor_mul(out=yT_cs[c][0:64, mt, :],
                                         in0=y_sb[0:64, :], in1=bc)
                else:
                    yt = wk.tile([64, LQC], F32R, tag="yt", bufs=1, name="yt")
                    nc.vector.tensor_mul(out=yt, in0=y_sb[0:64, :], in1=bc)
                    nc.sync.dma_start(out=yT_cs[c][64:128, mt, :], in_=yt)

            P = {}   # late-bound tile pools: 'sc', 'pv', 'pp'

            def att_head(c, h, fillers, fill_points):
                scp, pvp = P['sc'], P['pv']
                base, mt, j = 64 * (h // 4), h % 4, h // 4
                ntile = 8 * (c + 1)
                pv = pvp.tile([65, LQC], F32, tag="pv", name="pv")
                pend_pv = []   # lag-2 software pipeline: pv trails scores
                for t in range(ntile):
                    os_, oe, masked = seg_info(c, t)
                    segs = segs_of(os_)
                    sc_t = scp.tile([128, LQC], F32, tag="sc", name="sc_t")
                    for (a, b) in segs:
                        nc.tensor.matmul(
                            sc_t[:, a:b],
                            kT_sb[base:base + 64, 128 * t:128 * t + 128],
                            qT_sb[base:base + 64, mt, LQC * c + a:LQC * c + b],
                            start=True, stop=True)
                    # causal mask: multiplicative 0/1 keep-mask on Pool,
                    # applied POST-exp on expS (SBUF — gpsimd cannot touch
                    # PSUM). Keeps the score->exp chain two-engine so ACT
                    # never waits on a mask op; the PV matmul eats the mask
                    # latency where the PE has filler slack.
                    ex = wk.tile([128, LQC], F32R, tag="expS", bufs=3, name="ex")
                    nc.scalar.activation(out=ex[:, os_:LQC], in_=sc_t[:, os_:LQC],
                                         func=EXP, scale=0.125)
                    if masked:
                        if oe == os_:
                            nc.gpsimd.tensor_mul(out=ex[:, oe:oe + 128],
                                                 in0=ex[:, oe:oe + 128],
                                                 in1=tri1_sb)
                        else:
                            nc.gpsimd.tensor_mul(out=ex[:, os_:os_ + 256],
                                                 in0=ex[:, os_:os_ + 256],
                                                 in1=tri2_sb)
                    pend_pv.append((ex, segs, t))
                    if len(pend_pv) > 2:
                        pex, psegs, pt = pend_pv.pop(0)
                        for (a, b) in psegs:
                            nc.tensor.matmul(pv[:, a:b],
                                             v_aug[:, pt, 65 * j:65 * j + 65],
                                             pex[:, a:b],
                                             start=(pt == 0), stop=False)
                    if t == 0:
                        norm_part2()
                    if t in fill_points and fillers:
                        fillers.pop(0)()
                while pend_pv:
                    pex, psegs, pt = pend_pv.pop(0)
                    for (a, b) in psegs:
                        nc.tensor.matmul(pv[:, a:b],
                                         v_aug[:, pt, 65 * j:65 * j + 65],
                                         pex[:, a:b],
                                         start=(pt == 0),
                                         stop=(pt == ntile - 1))
                norm_part1(c, h, pv)

            # ---- Phase A projections + c0/c1 attention + out_proj ----
            pend_tr = []

            def proj_group(n, f, bias_dve=False):
                """f: 0=k, 1=v, 2..5 = q tile f-2. 16 matmuls + bias.
                V transposes are deferred to the next group so the PE
                never waits on the bias-add. bias_dve routes the bias to
                DVE (used for c0-filler chunks where ACT is exp-bound)."""
                pp = P['pp']
                x_sb = x_tiles[n]
                sl = slice(CHA * n, CHA * n + CHA)
                ps = pp.tile([128, 512], F32, tag="pp", name="ps")
                for kt in range(NKT):
                    if f == 0:
                        stat = kw_sb[:, kt, :]
                    elif f == 1:
                        stat = vw_sb[:, kt, :]
                    else:
                        mt = f - 2
                        stat = qw_sb[:, kt, 128 * mt:128 * mt + 128]
                    nc.tensor.matmul(ps[:, 0:CHA], stat, x_sb[:, kt, :],
                                     start=(kt == 0), stop=(kt == NKT - 1))

                def bias_add(out_ap, bias_ap):
                    if bias_dve:
                        nc.vector.tensor_scalar_add(out=out_ap, in0=ps[:, 0:CHA],
                                                    scalar1=bias_ap)
                    else:
                        nc.scalar.activation(out=out_ap, in_=ps[:, 0:CHA],
                                             func=IDENT, bias=bias_ap, scale=1.0)
                if f == 0:
                    bias_add(kT_sb[:, sl], kb_sb)
                elif f >= 2:
                    mt = f - 2
                    bias_add(qT_sb[:, mt, sl], qb_sb[:, mt:mt + 1])
                else:
                    vtmp = vt.tile([128, CHA], F32R, tag="vt", name="vtmp")
                    bias_add(vtmp, vb_sb)
                while pend_tr:
                    pend_tr.pop(0)()
                if f == 1:
                    vtmp_ = vtmp
                    for tt in range(CHA // 128):
                        def tr(n=n, tt=tt, vtmp=vtmp_):
                            trpool = P['pp']
                            ti = (CHA * n) // 128 + tt
                            trp = trpool.tile([128, 512], F32R, tag="pp",
                                              name="trp")
                            nc.tensor.transpose(
                                trp[:, 0:128],
                                vtmp[:, 128 * tt:128 * tt + 128], idn)
                            nc.vector.tensor_copy(out=v_aug[:, ti, 0:64],
                                                  in_=trp[:, 0:64])
                            nc.vector.tensor_copy(out=v_aug[:, ti, 65:129],
                                                  in_=trp[:, 64:128])
                        pend_tr.append(tr)

            order = [1, 0, 2, 3, 4, 5]   # v first, then k, then q tiles

            # projections for tokens 0:1024 (chunks 0..3) in a dedicated
            # 4-bank PSUM pool: deep rotation keeps the PE stream gapless
            # so the p-state ramp reaches full clock. v/k groups of chunks
            # 0-1 run first so the PE is never waiting on the large qw DMA.
            with tc.tile_pool(name="ppA", bufs=4, space="PSUM") as ppA:
                P['pp'] = ppA
                for n in (0, 1):
                    proj_group(n, 1)
                    proj_group(n, 0)
                for f in (2, 3, 4, 5):
                    proj_group(0, f)
                x_dma(2)
                for f in (2, 3, 4, 5):
                    proj_group(1, f)
                proj_group(2, 1)
                proj_group(2, 0)
                x_dma(3)
                for f in (2, 3, 4, 5):
                    proj_group(2, f)
                proj_group(3, 1)
                proj_group(3, 0)
                x_dma(4)
                for f in (2, 3, 4, 5):
                    proj_group(3, f)
                x_dma(5)
                while pend_tr:
                    pend_tr.pop(0)()

            scp = tc.alloc_tile_pool(name="scp", bufs=2, space="PSUM")
            pvp = tc.alloc_tile_pool(name="pvp", bufs=1, space="PSUM")
            P['sc'], P['pv'] = scp, pvp

            # c0 attention with projection chunks 4..7 as PE filler
            # (filler biases on DVE: ACT is exp-bound during attention)
            with tc.tile_pool(name="ppB", bufs=2, space="PSUM") as ppB:
                P['pp'] = ppB
                fillers = []
                for n in range(4, NCHA):
                    for i, f in enumerate(order):
                        def g(n=n, f=f, last=(i == len(order) - 1)):
                            proj_group(n, f, bias_dve=True)
                            if last and n + 2 < NCHA:
                                x_dma(n + 2)
                        fillers.append(g)
                # heads 4-7 first: their normalize needs an extra shift
                # DMA, so the LAST heads' (0-3) normalize chains are short
                # and the downstream out_proj starts sooner
                for h in (4, 5, 6, 7, 0, 1, 2, 3):
                    att_head(0, h, fillers, fill_points=(1, 3, 5))
                while fillers:
                    fillers.pop(0)()
                while pend_tr:
                    pend_tr.pop(0)()
            vt.release()
            xp.release()

            # ---- c1 attention with out_proj chunk-0 filler; tail ----
            owp = tc.alloc_tile_pool(name="owp", bufs=1, space="SBUF")
            od = tc.alloc_tile_pool(name="od", bufs=1, space="SBUF")
            opsC1 = tc.alloc_tile_pool(name="opsC1", bufs=2, space="PSUM")
            P['ops'] = opsC1

            ow_tiles = {}

            def ow_dma(ot):
                ow_t = owp.tile([128, 4, 128], F32R, tag="ow", bufs=16,
                                name="ow_t")
                nc.sync.dma_start(out=ow_t,
                                  in_=ow.ap()[:, :, 128 * ot:128 * ot + 128])
                ow_tiles[ot] = ow_t

            def d_group(ot, half, pool_copy=False, split_store=False):
                """8 matmuls + 2 psum copies + ONE [128,1024] store
                for output token range half*1024 : half*1024+1024.
                split_store streams each 512-half as soon as it is copied
                (used for the final groups to shorten the drain tail)."""
                ow_t = ow_tiles[ot]
                o_sb = od.tile([128, 1024], F32, tag="osb", bufs=4,
                               name="o_sb")
                for sub in range(2):
                    o_ps = P['ops'].tile([128, 512], F32, tag="ops",
                                         name="o_ps")
                    for it in range(4):
                        nc.tensor.matmul(
                            o_ps, ow_t[:, it, :],
                            yT_cs[half][:, it, 512 * sub:512 * sub + 512],
                            start=(it == 0), stop=(it == 3))
                    # gpsimd can't read PSUM; ACT is idle in the tail, so
                    # alternate the psum drains between DVE and ACT there
                    if pool_copy and sub == 1:
                        nc.scalar.copy(out=o_sb[:, 512 * sub:512 * sub + 512],
                                       in_=o_ps)
                    else:
                        nc.vector.tensor_copy(
                            out=o_sb[:, 512 * sub:512 * sub + 512], in_=o_ps)
                    if split_store:
                        nc.sync.dma_start(
                            out=outp.ap()[ot, :,
                                          1024 * half + 512 * sub:
                                          1024 * half + 512 * sub + 512],
                            in_=o_sb[:, 512 * sub:512 * sub + 512])
                if not split_store:
                    nc.sync.dma_start(
                        out=outp.ap()[ot, :, 1024 * half:1024 * half + 1024],
                        in_=o_sb)

            fillers = []
            for ot in range(NKT):
                def g0(ot=ot):
                    if ot == 0:
                        ow_dma(0)
                        ow_dma(1)
                    if ot + 2 < NKT:
                        ow_dma(ot + 2)
                    d_group(ot, 0)
                fillers.append(g0)
            for h in (4, 5, 6, 7, 0, 1, 2, 3):
                att_head(1, h, fillers, fill_points=(1, 9))
            while fillers:
                fillers.pop(0)()
            norm_part2()
            norm_part2()

            # tail: attention PSUM pools are done — hand their 6 banks to
            # a deep out_proj accumulation pool so nothing rotates-stalls
            opsC1.release()
            pvp.release()
            scp.release()
            opsD2 = tc.alloc_tile_pool(name="opsD2", bufs=6, space="PSUM")
            P['ops'] = opsD2
            for ot in range(NKT):
                d_group(ot, 1, pool_copy=True, split_store=(ot >= NKT - 2))
            opsD2.release()
            od.release()
            owp.release()
    nc.compile()
    return nc


def _perm512():
    p = np.empty(512, dtype=np.int64)
    for mt in range(4):
        for half in range(2):
            head = mt + 4 * half
            p[128 * mt + 64 * half:128 * mt + 64 * half + 64] = \
                np.arange(64 * head, 64 * head + 64)
    return p


def kernel(x, attention_mask, q_w, q_b, k_w, k_b, v_w, v_b, o_w, o_b):
    from concourse.bass_utils import run_bass_kernel_spmd

    x = np.asarray(x, dtype=np.float32)
    q_w = np.asarray(q_w, dtype=np.float32); q_b = np.asarray(q_b, dtype=np.float32)
    k_w = np.asarray(k_w, dtype=np.float32); k_b = np.asarray(k_b, dtype=np.float32)
    v_w = np.asarray(v_w, dtype=np.float32); v_b = np.asarray(v_b, dtype=np.float32)
    o_w = np.asarray(o_w, dtype=np.float32); o_b = np.asarray(o_b, dtype=np.float32)
    am = np.asarray(attention_mask)
    assert am.all(), "kernel assumes attention_mask == all ones"

    if "nc" not in _cached:
        _cached["nc"] = _build()
    nc = _cached["nc"]

    perm = _perm512()
    # 0/1 keep-masks applied post-exp: 0 where lk > lq (causal), 1 elsewhere
    causal = np.arange(128)[:, None] > np.arange(128)[None, :]
    tri1_np = np.where(causal, np.float32(0), np.float32(1)).astype(np.float32)
    tri2_np = np.concatenate(
        [np.zeros((128, 128), dtype=np.float32), tri1_np], axis=1)
    id_np = np.eye(128, dtype=np.float32)

    in_maps = []
    for c in range(NCORE):
        b, g = c // 4, c % 4
        G0 = 512 * g
        xT_t = np.ascontiguousarray(
            x[b].T.reshape(NKT, 128, L).transpose(1, 0, 2))
        qws = q_w[G0:G0 + 512][perm]
        qw_t = np.ascontiguousarray(qws.T.reshape(NKT, 128, 512).transpose(1, 0, 2))
        kws = k_w[128 * g:128 * g + 128]
        kw_t = np.ascontiguousarray(kws.T.reshape(NKT, 128, 128).transpose(1, 0, 2))
        vws = v_w[128 * g:128 * g + 128]
        vw_t = np.ascontiguousarray(vws.T.reshape(NKT, 128, 128).transpose(1, 0, 2))
        owp = o_w[:, G0:G0 + 512][:, perm]
        ow_t = np.ascontiguousarray(owp.T.reshape(4, 128, HID).transpose(1, 0, 2))
        qb_t = np.ascontiguousarray(q_b[G0:G0 + 512][perm].reshape(4, 128).T)
        kb_t = k_b[128 * g:128 * g + 128].reshape(128, 1).copy()
        vb_t = v_b[128 * g:128 * g + 128].reshape(128, 1).copy()
        in_maps.append({"xT": xT_t, "qw": qw_t, "kw": kw_t, "vw": vw_t,
                        "ow": ow_t, "qb": qb_t, "kb": kb_t, "vb": vb_t,
                        "tri1": tri1_np, "tri2": tri2_np, "ident": id_np})

    res = run_bass_kernel_spmd(nc, in_maps, core_ids=list(range(NCORE)))
    out = np.empty((2, L, HID), dtype=np.float32)
    for b in range(2):
        acc = res.results[4 * b]["outp"].astype(np.float32).copy()
        for i in range(1, 4):
            acc += res.results[4 * b + i]["outp"]
        out[b] = acc.reshape(HID, L).T + o_b
    return out
